# revision 1
# baseline (speedup 1.0000x reference)
"""Trainium2 Bass kernel for HeadTailBoundaryPredictor.

Reference computation (B=8, S=512, E=16, H=768):
    t   = token @ Wt.T + bt                    [B,S,H]
    e2  = ent @ We.T + be                      [B,E,H]
    cls = einsum('besh,h->bes', relu(t[:,None]+e2[:,:,None]), wb)
    cls = where(mask, cls, -1e4); p = sigmoid(cls)

Math restructure: fold wb into the projections. With a = |wb|, s = sign(wb):
    cls[e,s] = sum_o s[o] * relu( a[o]*t[s,o] + a[o]*e2[e,o] )
since a[o]*relu(x) = relu(a[o]*x) for a >= 0. So the device computes
    u'T[o,s]  = (diag(a) Wt token[b].T)           via TensorE (fp32r)
    v''T[o,e] = (diag(a) We entT + a*(bt+be))     via TensorE (fp32r)
    act[o,s]  = relu(u'T + v''T[:,e])             per entity, per-partition bias
                (VectorE tensor_scalar / ScalarE activation, fp32)
    cls[e,s]  = sgn.T @ act                       column-tiled (128x32) matmuls,
                                                  4 entities concurrent, fp32r
    cls_m = min(cls, Lmask); p = sigmoid(cls_m)

Sharding: data-parallel over batch B across 8 cores; weights replicated.
"""

import sys

for _p in ("/opt/trn_rl_repo", "/root/.axon_site/_ro/trn_rl_repo"):
    if _p not in sys.path:
        sys.path.append(_p)

import numpy as np

import concourse.bass as bass
import concourse.mybir as mybir
import concourse.tile as tile
from concourse.bass_utils import run_bass_kernel_spmd

dt = mybir.dt
AF = mybir.ActivationFunctionType
ALU = mybir.AluOpType

B, S, E, H = 8, 512, 16, 768
P = 128
NH = H // P  # 6 chunks of the hidden/output dims
NS = S // P  # 4 s-chunks
NQ = E // 4  # 4 quads of entities

N_DVE_ACTS = 11  # entities 0..10 on VectorE, 11..15 on ScalarE

_WAITSPLIT_CTR = [0]


def _split_excess_waits(nc, limit=1):
    """walrus (CoreV3) accepts at most `limit` sync-wait commands per
    instruction; Tile can emit more (e.g. the tail drain). Move excess waits
    onto freshly inserted same-engine NoOps, which is semantically identical."""
    n = 0
    for f in nc.m.functions:
        for bb in f.blocks:
            insts = list(bb.instructions)
            out = []
            changed = False
            for inst in insts:
                si = inst.sync_info
                waits = list(si.on_wait) if si else []
                if len(waits) > limit:
                    head, tail = waits[:-limit], waits[-limit:]
                    for i in range(0, len(head), limit):
                        _WAITSPLIT_CTR[0] += 1
                        nop = mybir.InstNoOp(
                            name=f"waitsplit_nop_{_WAITSPLIT_CTR[0]}", ins=[], outs=[]
                        )
                        nop.engine = inst.engine
                        nop.sync_info = mybir.SyncInfo(
                            on_wait=head[i : i + limit], on_update=[]
                        )
                        out.append(nop)
                        n += 1
                    si.on_wait = tail
                    inst.sync_info = si
                    changed = True
                out.append(inst)
            if changed:
                bb.instructions = out
    return n


def _build_nc():
    nc = bass.Bass()

    tokenT = nc.dram_tensor("tokenT", [H, S], dt.float32r, kind="ExternalInput")
    wtT = nc.dram_tensor("wtT", [H, H], dt.float32r, kind="ExternalInput")
    weT = nc.dram_tensor("weT", [H, H], dt.float32r, kind="ExternalInput")
    entT = nc.dram_tensor("entT", [H, E], dt.float32r, kind="ExternalInput")
    sgnT = nc.dram_tensor("sgnT", [P, NH], dt.float16, kind="ExternalInput")
    bbT = nc.dram_tensor("bbT", [P, NH], dt.float32, kind="ExternalInput")
    lmask = nc.dram_tensor("lmask", [1, S], dt.float32, kind="ExternalInput")

    cls_out = nc.dram_tensor("cls_out", [E, S], dt.float32, kind="ExternalOutput")
    p_out = nc.dram_tensor("p_out", [E, S], dt.float32, kind="ExternalOutput")

    with tile.TileContext(nc) as tc:
        with (
            tc.tile_pool(name="const", bufs=1) as cpool,
            tc.tile_pool(name="wts", bufs=1) as wpool,
            tc.tile_pool(name="usb", bufs=NH) as upool,
            tc.tile_pool(name="acts", bufs=48) as apool,
            tc.tile_pool(name="outs", bufs=1) as opool,
            tc.tile_pool(name="psv", bufs=1, space="PSUM") as psv,
            tc.tile_pool(name="psu", bufs=2, space="PSUM") as psu,
            tc.tile_pool(name="psc", bufs=1, space="PSUM") as psc,
        ):
            # ---- small constants first (also feed the PE warmup) ----
            t_sgn = cpool.tile([P, NH], dt.float16, tag="sgn")
            nc.sync.dma_start(t_sgn[:], sgnT[:])
            t_bb = cpool.tile([P, NH], dt.float32, tag="bb")
            nc.sync.dma_start(t_bb[:], bbT[:])
            t_lb = cpool.tile([P, S], dt.float32, tag="lb")
            nc.sync.dma_start(t_lb[:], lmask[:].partition_broadcast(P))

            # dummy sigmoid to pull the ACT table load off the critical path
            t_dummy = cpool.tile([P, 1], dt.float32, tag="dmy")
            nc.scalar.activation(t_dummy[:], t_bb[:, 0:1], AF.Sigmoid)

            # ---- big input DMAs (per h-chunk so compute can start early) ----
            t_wt = wpool.tile([P, NH * H], dt.float32r, tag="wt")
            t_we = wpool.tile([P, NH * H], dt.float32r, tag="we")
            t_tok = wpool.tile([P, NH * S], dt.float32r, tag="tok")
            t_ent = wpool.tile([P, NH * E], dt.float32r, tag="ent")
            for k in range(NH):
                nc.sync.dma_start(t_wt[:, k * H : (k + 1) * H], wtT[k * P : (k + 1) * P, :])
                nc.sync.dma_start(t_tok[:, k * S : (k + 1) * S], tokenT[k * P : (k + 1) * P, :])
            for k in range(NH):
                nc.sync.dma_start(t_we[:, k * H : (k + 1) * H], weT[k * P : (k + 1) * P, :])
                nc.sync.dma_start(
                    t_ent[:, k * E : (k + 1) * E], entT[k * P : (k + 1) * P, :]
                )

            # ---- PE warmup against the HAM clock gate (scratch psum bank) ----
            ps_w = psv.tile([P, P], dt.float32, tag="vps")
            for w in range(64):
                nc.tensor.matmul(
                    ps_w[0:NH, 0:NH], t_sgn[:], t_sgn[:], start=True, stop=True
                )

            # ---- entity projection: v''T[o, e] for this core's batch ----
            v_sb = cpool.tile([P, NH * E], dt.float32, tag="vsb")
            for j in range(NH):
                ps_v = psv.tile([P, E], dt.float32, tag="vps", name=f"ps_v{j}")
                for k in range(NH):
                    nc.tensor.matmul(
                        ps_v[:],
                        t_we[:, k * H + j * P : k * H + (j + 1) * P],
                        t_ent[:, k * E : (k + 1) * E],
                        start=(k == 0),
                        stop=(k == NH - 1),
                    )
                # copy to SBUF and add the folded bias a*(bt+be) per partition
                nc.vector.tensor_scalar(
                    v_sb[:, j * E : (j + 1) * E],
                    ps_v[:],
                    t_bb[:, j : j + 1],
                    None,
                    op0=ALU.add,
                )

            # ---- main pipeline over o-chunks j ----
            u_sb = [None] * NH
            act_t = [[None] * E for _ in range(NH)]
            ps_c = [psc.tile([P, S], dt.float32, tag=f"cq{q}", name=f"ps_c{q}") for q in range(NQ)]

            def uproj(j):
                ps_u = psu.tile([P, S], dt.float32, tag="ups", name=f"ps_u{j}")
                for k in range(NH):
                    nc.tensor.matmul(
                        ps_u[:],
                        t_wt[:, k * H + j * P : k * H + (j + 1) * P],
                        t_tok[:, k * S : (k + 1) * S],
                        start=(k == 0),
                        stop=(k == NH - 1),
                    )
                u_sb[j] = upool.tile([P, S], dt.float32, tag="u", name=f"u_sb{j}")
                nc.scalar.copy(u_sb[j][:], ps_u[:])

            def acts(j):
                for e in range(E):
                    a = apool.tile([P, S], dt.float16, tag="act", name=f"act_{j}_{e}")
                    act_t[j][e] = a
                    bias = v_sb[:, j * E + e : j * E + e + 1]
                    if e < N_DVE_ACTS:
                        nc.vector.tensor_scalar(
                            a[:], u_sb[j][:], bias, 0.0, op0=ALU.add, op1=ALU.max
                        )
                    else:
                        nc.scalar.activation(a[:], u_sb[j][:], AF.Relu, bias=bias)

            def reduce(j):
                for q in range(NQ):
                    for g in range(4):
                        e = 4 * q + g
                        nc.tensor.matmul(
                            ps_c[q][32 * g : 32 * g + 1, :],
                            t_sgn[:, j : j + 1],
                            act_t[j][e][:],
                            start=(j == 0),
                            stop=(j == NH - 1),
                            tile_position=(0, 32 * g),
                        )

            # software-pipelined issue order: reduce lags uproj by 2 chunks
            for j in range(NH):
                uproj(j)
                acts(j)
                if j >= 2:
                    reduce(j - 2)
            reduce(NH - 2)
            reduce(NH - 1)

            # ---- mask + sigmoid + write out ----
            for q in range(NQ):
                cls_sb = opool.tile([P, S], dt.float32, tag=f"cls{q}", name=f"cls_sb{q}")
                nc.vector.tensor_tensor(cls_sb[:], ps_c[q][:], t_lb[:], op=ALU.min)
                p_sb = opool.tile([P, S], dt.float32, tag=f"p{q}", name=f"p_sb{q}")
                nc.scalar.activation(p_sb[:], cls_sb[:], AF.Sigmoid)
                nc.sync.dma_start(cls_out[4 * q : 4 * q + 4, :], cls_sb[0:P:32, :])
                nc.sync.dma_start(p_out[4 * q : 4 * q + 4, :], p_sb[0:P:32, :])

    _split_excess_waits(nc, limit=1)
    return nc


_NC_CACHE = {}


def _get_nc():
    if "nc" not in _NC_CACHE:
        _NC_CACHE["nc"] = _build_nc()
    return _NC_CACHE["nc"]


def kernel(token_embedding, entity_embedding, token_mask, Wt, bt, We, be, wb, **kw):
    token_embedding = np.asarray(token_embedding, dtype=np.float32)
    entity_embedding = np.asarray(entity_embedding, dtype=np.float32)
    token_mask = np.asarray(token_mask)
    Wt = np.asarray(Wt, dtype=np.float32)
    bt = np.asarray(bt, dtype=np.float32)
    We = np.asarray(We, dtype=np.float32)
    be = np.asarray(be, dtype=np.float32)
    wb = np.asarray(wb, dtype=np.float32)

    a = np.abs(wb)
    sgn = np.where(wb >= 0, np.float32(1.0), np.float32(-1.0))

    wtT2 = np.ascontiguousarray((Wt * a[:, None]).T, dtype=np.float32)  # [h, o]
    weT2 = np.ascontiguousarray((We * a[:, None]).T, dtype=np.float32)
    bb = ((bt + be) * a).astype(np.float32)
    bbT = np.ascontiguousarray(bb.reshape(NH, P).T)  # [128, 6]
    sgnT = np.ascontiguousarray(sgn.reshape(NH, P).T).astype(np.float16)  # [128, 6]

    nc = _get_nc()
    in_maps = []
    for b in range(B):
        lm = np.where(token_mask[b], np.float32(1e30), np.float32(-1e4)).astype(
            np.float32
        )[None, :]
        in_maps.append(
            {
                "tokenT": np.ascontiguousarray(token_embedding[b].T),
                "wtT": wtT2,
                "weT": weT2,
                "entT": np.ascontiguousarray(entity_embedding[b].T),
                "sgnT": sgnT,
                "bbT": bbT,
                "lmask": lm,
            }
        )

    res = run_bass_kernel_spmd(nc, in_maps, core_ids=list(range(B)))

    cls = np.stack([res.results[b]["cls_out"] for b in range(B)])
    p = np.stack([res.results[b]["p_out"] for b in range(B)])
    return cls, p



# revision 5
# speedup vs baseline: 1.0609x; 1.0609x over previous
"""Trainium2 Bass kernel for HeadTailBoundaryPredictor.

Reference computation (B=8, S=512, E=16, H=768):
    t   = token @ Wt.T + bt                    [B,S,H]
    e2  = ent @ We.T + be                      [B,E,H]
    cls = einsum('besh,h->bes', relu(t[:,None]+e2[:,:,None]), wb)
    cls = where(mask, cls, -1e4); p = sigmoid(cls)

Math restructure: fold wb into the projections. With a = |wb|, s = sign(wb):
    cls[e,s] = sum_o s[o] * relu( a[o]*t[s,o] + a[o]*e2[e,o] )
since a[o]*relu(x) = relu(a[o]*x) for a >= 0.

Device plan (per core = one batch, data-parallel over B):
  - Host compacts the sequence dim: only token positions with mask=1 are
    shipped/computed (S_c = roundup(max_count, 64)); masked outputs are the
    constants -1e4 / sigmoid(-1e4)=0, filled host-side.
  - token/Wt/We/ent are bf16 (halves DMA); u/acts are f16 so the DVE runs
    tensor_scalar in 4x mode; reduce matmuls are f16 (full PE rate).
  - u'T[o,s]  = (diag(a) Wt tokenT)   per o-chunk j, bf16 matmuls (TensorE)
  - v''T[o,e] = (diag(a) We entT) + a*(bt+be)   (TensorE + bias add)
  - act[o,s]  = relu(u' + v''[:,e]) f16, per entity: 13 on VectorE (4x mode),
    2 on ScalarE, 1 on GpSimd
  - cls[e,s]  = sgnT @ act   via 1-column f16 matmuls rotated over the 4
    PE column groups (tile_position) so up to 4 run concurrently
  - p = sigmoid(cls) (ScalarE); outputs f16, host casts/scatters.
  - All DRAM inputs are host-packed partition-major so each is a single
    contiguous-per-partition DMA; descriptor gen is split across the
    SP and ACT hardware DGE queues (weights j-sliced to unblock compute).
"""

import sys

for _p in ("/opt/trn_rl_repo", "/root/.axon_site/_ro/trn_rl_repo"):
    if _p not in sys.path:
        sys.path.append(_p)

import numpy as np
import ml_dtypes

import concourse.bass as bass
import concourse.mybir as mybir
import concourse.tile as tile
from concourse.bass_utils import run_bass_kernel_spmd

dt = mybir.dt
AF = mybir.ActivationFunctionType
ALU = mybir.AluOpType

B, S, E, H = 8, 512, 16, 768
P = 128
NH = H // P  # 6 chunks of the hidden/output dims
NQ = E // 4  # 4 entity quads (one PSUM bank each)

N_WARMUP = 16
N_DVE = 13  # entities 0..12 on VectorE, 13..14 on ScalarE, 15 on GpSimd

_WAITSPLIT_CTR = [0]


def _split_excess_waits(nc, limit=1):
    """walrus (CoreV3) accepts at most `limit` sync-wait commands per
    instruction; Tile can emit more (e.g. the tail drain). Move excess waits
    onto freshly inserted same-engine NoOps, which is semantically identical."""
    n = 0
    for f in nc.m.functions:
        for bb in f.blocks:
            insts = list(bb.instructions)
            out = []
            changed = False
            for inst in insts:
                si = inst.sync_info
                waits = list(si.on_wait) if si else []
                if len(waits) > limit:
                    head, tail = waits[:-limit], waits[-limit:]
                    for i in range(0, len(head), limit):
                        _WAITSPLIT_CTR[0] += 1
                        nop = mybir.InstNoOp(
                            name=f"waitsplit_nop_{_WAITSPLIT_CTR[0]}", ins=[], outs=[]
                        )
                        nop.engine = inst.engine
                        nop.sync_info = mybir.SyncInfo(
                            on_wait=head[i : i + limit], on_update=[]
                        )
                        out.append(nop)
                        n += 1
                    si.on_wait = tail
                    inst.sync_info = si
                    changed = True
                out.append(inst)
            if changed:
                bb.instructions = out
    return n


def _build_nc(S_c):
    nc = bass.Bass()

    tok_pk = nc.dram_tensor("tok_pk", [P, NH * S_c], dt.bfloat16, kind="ExternalInput")
    wt_pk = nc.dram_tensor("wt_pk", [P, NH * NH * P], dt.bfloat16, kind="ExternalInput")
    we_pk = nc.dram_tensor("we_pk", [P, NH * NH * P], dt.bfloat16, kind="ExternalInput")
    ent_pk = nc.dram_tensor("ent_pk", [P, NH * E], dt.bfloat16, kind="ExternalInput")
    sgn_pk = nc.dram_tensor("sgn_pk", [P, NH], dt.float16, kind="ExternalInput")
    bb_pk = nc.dram_tensor("bb_pk", [P, NH], dt.float32, kind="ExternalInput")

    # out[q, g, c]: entity e = 4q+g; c = [cls | p] each S_c wide
    out_t = nc.dram_tensor("out", [NQ, 4, 2 * S_c], dt.float16, kind="ExternalOutput")

    with tile.TileContext(nc) as tc:
        with (
            tc.tile_pool(name="const", bufs=1) as cpool,
            tc.tile_pool(name="wts", bufs=1) as wpool,
            tc.tile_pool(name="usb", bufs=NH) as upool,
            tc.tile_pool(name="acts", bufs=48) as apool,
            tc.tile_pool(name="outs", bufs=1) as opool,
            tc.tile_pool(name="psw", bufs=1, space="PSUM") as psw,
            tc.tile_pool(name="psv", bufs=1, space="PSUM") as psv,
            tc.tile_pool(name="psu", bufs=2, space="PSUM") as psu,
            tc.tile_pool(name="psc", bufs=1, space="PSUM") as psc,
        ):
            # ---- SBUF tiles ----
            t_sgn = cpool.tile([P, NH], dt.float16, tag="sgn")
            t_bb = cpool.tile([P, NH], dt.float32, tag="bb")
            t_dmy = cpool.tile([P, 1], dt.float32, tag="dmy")
            t_wscr = cpool.tile([P, 256], dt.float16, tag="wscr")
            v_sb = cpool.tile([P, NH * E], dt.float32, tag="vsb")
            t_wt = wpool.tile([P, NH * NH * P], dt.bfloat16, tag="wt")
            t_we = wpool.tile([P, NH * NH * P], dt.bfloat16, tag="we")
            t_tok = wpool.tile([P, NH * S_c], dt.bfloat16, tag="tok")
            t_ent = wpool.tile([P, NH * E], dt.bfloat16, tag="ent")
            osb = opool.tile([P, NQ * 2 * S_c], dt.float16, tag="osb")

            # ---- DMA issue. ACT queue: consts + We (j-sliced) ----
            nc.scalar.dma_start(t_sgn[:], sgn_pk[:])
            nc.scalar.dma_start(t_bb[:], bb_pk[:])
            for j in range(3):
                sl = slice(j * NH * P, (j + 1) * NH * P)
                nc.scalar.dma_start(t_we[:, sl], we_pk[:, sl])
            # SP queue: token + Wt (j-sliced)
            nc.sync.dma_start(t_tok[:], tok_pk[:])
            for j in range(NH):
                sl = slice(j * NH * P, (j + 1) * NH * P)
                nc.sync.dma_start(t_wt[:, sl], wt_pk[:, sl])
            # GpSimd queue (SWDGE): scratch init, entities, tail of We
            nc.gpsimd.memset(t_wscr[:], 0.0)
            nc.gpsimd.dma_start(t_ent[:], ent_pk[:])
            for j in range(3, NH):
                sl = slice(j * NH * P, (j + 1) * NH * P)
                nc.gpsimd.dma_start(t_we[:, sl], we_pk[:, sl])

            # dummy sigmoid pulls the ACT table load off the critical path
            nc.scalar.activation(t_dmy[:], t_bb[:, 0:1], AF.Sigmoid)

            # ---- PE warmup (p-state ramp) on a scratch PSUM bank ----
            ps_w = psw.tile([P, 256], dt.float32, tag="wps")
            for w in range(N_WARMUP):
                nc.tensor.matmul(
                    ps_w[0:1, :], t_sgn[:, 0:1], t_wscr[:], start=True, stop=True
                )

            # ---- main pipeline ----
            ps_v = psv.tile([P, NH * E], dt.float32, tag="vps")
            u_sb = [None] * NH
            act_t = [[None] * E for _ in range(NH)]
            ps_c = [
                psc.tile([P, S_c], dt.float32, tag=f"cq{q}", name=f"ps_c{q}")
                for q in range(NQ)
            ]

            def vproj(j):
                # v''T[o in chunk j, e] accumulated over k; bias added on copy
                for k in range(NH):
                    nc.tensor.matmul(
                        ps_v[:, j * E : (j + 1) * E],
                        t_we[:, (j * NH + k) * P : (j * NH + k + 1) * P],
                        t_ent[:, k * E : (k + 1) * E],
                        start=(k == 0),
                        stop=(k == NH - 1),
                    )
                nc.vector.tensor_scalar(
                    v_sb[:, j * E : (j + 1) * E],
                    ps_v[:, j * E : (j + 1) * E],
                    t_bb[:, j : j + 1],
                    None,
                    op0=ALU.add,
                )

            def uproj(j):
                ps_u = psu.tile([P, S_c], dt.float32, tag="ups", name=f"ps_u{j}")
                for k in range(NH):
                    nc.tensor.matmul(
                        ps_u[:],
                        t_wt[:, (j * NH + k) * P : (j * NH + k + 1) * P],
                        t_tok[:, k * S_c : (k + 1) * S_c],
                        start=(k == 0),
                        stop=(k == NH - 1),
                    )
                u_sb[j] = upool.tile([P, S_c], dt.float16, tag="u", name=f"u_sb{j}")
                nc.scalar.copy(u_sb[j][:], ps_u[:])

            def acts(j):
                for e in range(E):
                    a = apool.tile([P, S_c], dt.float16, tag="act", name=f"act_{j}_{e}")
                    act_t[j][e] = a
                    bias = v_sb[:, j * E + e : j * E + e + 1]
                    if e < N_DVE:
                        nc.vector.tensor_scalar(
                            a[:], u_sb[j][:], bias, 0.0, op0=ALU.add, op1=ALU.max
                        )
                    elif e < N_DVE + 2:
                        nc.scalar.activation(a[:], u_sb[j][:], AF.Relu, bias=bias)
                    else:
                        nc.gpsimd.tensor_scalar(
                            a[:], u_sb[j][:], bias, 0.0, op0=ALU.add, op1=ALU.max
                        )

            def reduce(j):
                for e in range(E):
                    q, g = e // 4, e % 4
                    nc.tensor.matmul(
                        ps_c[q][32 * g : 32 * g + 1, :],
                        t_sgn[:, j : j + 1],
                        act_t[j][e][:],
                        start=(j == 0),
                        stop=(j == NH - 1),
                        tile_position=(0, 32 * g),
                    )

            # PE program order: warmup, then per-j v/u interleaved with
            # reduce lagging one chunk.
            vproj(0)
            uproj(0)
            acts(0)
            vproj(1)
            uproj(1)
            acts(1)
            reduce(0)
            for j in range(2, NH):
                vproj(j)
                uproj(j)
                acts(j)
                reduce(j - 1)
            reduce(NH - 1)

            # ---- tail: cls copy (DVE) + sigmoid (ACT) per quad, one out DMA
            for q in range(NQ):
                nc.vector.tensor_scalar(
                    osb[:, q * 2 * S_c : q * 2 * S_c + S_c],
                    ps_c[q][:],
                    0.0,
                    None,
                    op0=ALU.add,
                )
                nc.scalar.activation(
                    osb[:, q * 2 * S_c + S_c : (q + 1) * 2 * S_c],
                    ps_c[q][:],
                    AF.Sigmoid,
                )
            try:
                src = osb[0 : P : 32, :].rearrange("p (q c) -> p q c", q=NQ)
                dst = out_t[:].rearrange("q g c -> g q c")
                nc.sync.dma_start(dst, src)
            except Exception:
                for q in range(NQ):
                    nc.sync.dma_start(
                        out_t[q, :, :],
                        osb[0 : P : 32, q * 2 * S_c : (q + 1) * 2 * S_c],
                    )

    _split_excess_waits(nc, limit=1)
    return nc


_NC_CACHE = {}


def _get_nc(S_c):
    if S_c not in _NC_CACHE:
        _NC_CACHE[S_c] = _build_nc(S_c)
    return _NC_CACHE[S_c]


def _pack_pmajor(mat, ncols):
    """[H, ncols] -> [P, NH*ncols] partition-major: out[p, k*ncols+c] =
    mat[k*P+p, c]."""
    return np.ascontiguousarray(
        mat.reshape(NH, P, ncols).transpose(1, 0, 2).reshape(P, NH * ncols)
    )


def kernel(token_embedding, entity_embedding, token_mask, Wt, bt, We, be, wb, **kw):
    token_embedding = np.asarray(token_embedding, dtype=np.float32)
    entity_embedding = np.asarray(entity_embedding, dtype=np.float32)
    token_mask = np.asarray(token_mask).astype(bool)
    Wt = np.asarray(Wt, dtype=np.float32)
    bt = np.asarray(bt, dtype=np.float32)
    We = np.asarray(We, dtype=np.float32)
    be = np.asarray(be, dtype=np.float32)
    wb = np.asarray(wb, dtype=np.float32)

    bf16 = ml_dtypes.bfloat16

    a = np.abs(wb)
    sgn = np.where(wb >= 0, np.float32(1.0), np.float32(-1.0))

    # fold |wb| into the weights; transpose to [h, o]
    W2t = (Wt * a[:, None]).T.astype(np.float32)  # [h, o]
    W2e = (We * a[:, None]).T.astype(np.float32)
    bb = ((bt + be) * a).astype(np.float32)

    # wt_pk[p, (j*NH+k)*P + c] = W2[k*P+p, j*P+c]  (j-major blocks)
    def pack_w(W2):
        arr = W2.reshape(NH, P, NH, P).transpose(1, 2, 0, 3)  # [p, j, k, c]
        return np.ascontiguousarray(arr.reshape(P, NH * NH * P)).astype(bf16)

    wt_pk = pack_w(W2t)
    we_pk = pack_w(W2e)
    sgn_pk = np.ascontiguousarray(sgn.reshape(NH, P).T).astype(np.float16)
    bb_pk = np.ascontiguousarray(bb.reshape(NH, P).T).astype(np.float32)

    idxs = [np.nonzero(token_mask[b])[0] for b in range(B)]
    nmax = max((len(ix) for ix in idxs), default=1)
    S_c = max(64, -(-nmax // 64) * 64)

    nc = _get_nc(S_c)
    in_maps = []
    for b in range(B):
        ix = idxs[b]
        tokc = np.zeros((S_c, H), dtype=np.float32)
        tokc[: len(ix)] = token_embedding[b][ix]
        tok_pk = _pack_pmajor(tokc.T, S_c).astype(bf16)  # [P, NH*S_c]
        ent_pk = _pack_pmajor(entity_embedding[b].T, E).astype(bf16)
        in_maps.append(
            {
                "tok_pk": tok_pk,
                "wt_pk": wt_pk,
                "we_pk": we_pk,
                "ent_pk": ent_pk,
                "sgn_pk": sgn_pk,
                "bb_pk": bb_pk,
            }
        )

    res = run_bass_kernel_spmd(nc, in_maps, core_ids=list(range(B)))

    cls = np.full((B, E, S), -10000.0, dtype=np.float32)
    p = np.zeros((B, E, S), dtype=np.float32)
    for b in range(B):
        o = np.asarray(res.results[b]["out"], dtype=np.float32).reshape(E, 2 * S_c)
        ix = idxs[b]
        cls[b][:, ix] = o[:, : len(ix)]
        p[b][:, ix] = o[:, S_c : S_c + len(ix)]
    return cls, p


# revision 7
# speedup vs baseline: 1.5013x; 1.4152x over previous
"""Trainium2 Bass kernel for HeadTailBoundaryPredictor.

Reference computation (B=8, S=512, E=16, H=768):
    t   = token @ Wt.T + bt                    [B,S,H]
    e2  = ent @ We.T + be                      [B,E,H]
    cls = einsum('besh,h->bes', relu(t[:,None]+e2[:,:,None]), wb)
    cls = where(mask, cls, -1e4); p = sigmoid(cls)

Math restructure: fold wb into the projections. With a = |wb|, s = sign(wb):
    cls[e,s] = sum_o s[o] * relu( a[o]*t[s,o] + a[o]*e2[e,o] )
since a[o]*relu(x) = relu(a[o]*x) for a >= 0.

Device plan (per core = one batch, data-parallel over B):
  - Host compacts the sequence dim: only token positions with mask=1 are
    shipped/computed (S_c = roundup(max_count, 64)); masked outputs are the
    constants -1e4 / sigmoid(-1e4)=0, filled host-side.
  - token/Wt/We/ent are bf16 (halves DMA); u/acts are f16 so the DVE runs
    tensor_scalar in 4x mode; reduce matmuls are f16 (full PE rate).
  - u'T[o,s]  = (diag(a) Wt tokenT)   per o-chunk j, bf16 matmuls (TensorE)
  - v''T[o,e] = (diag(a) We entT) + a*(bt+be)   (TensorE + bias add)
  - act[o,s]  = relu(u' + v''[:,e]) f16, per entity: 13 on VectorE (4x mode),
    2 on ScalarE, 1 on GpSimd
  - cls[e,s]  = sgnT @ act   via 1-column f16 matmuls rotated over the 4
    PE column groups (tile_position) so up to 4 run concurrently
  - p = sigmoid(cls) (ScalarE); outputs f16, host casts/scatters.
  - All DRAM inputs are host-packed partition-major so each is a single
    contiguous-per-partition DMA; descriptor gen is split across the
    SP and ACT hardware DGE queues (weights j-sliced to unblock compute).
"""

import sys

for _p in ("/opt/trn_rl_repo", "/root/.axon_site/_ro/trn_rl_repo"):
    if _p not in sys.path:
        sys.path.append(_p)

import numpy as np
import ml_dtypes

import concourse.bass as bass
import concourse.mybir as mybir
import concourse.tile as tile
from concourse.bass_utils import run_bass_kernel_spmd

dt = mybir.dt
AF = mybir.ActivationFunctionType
ALU = mybir.AluOpType

B, S, E, H = 8, 512, 16, 768
P = 128
NH = H // P  # 6 chunks of the hidden/output dims
NQ = E // 4  # 4 entity quads (one PSUM bank each)

N_WARMUP = 12
N_DVE = 10  # entities 0..9 on VectorE (from u_sb f16); 10..15 on ScalarE (from PSUM)
# reduce consumption order: ScalarE entities are ready first (no u-copy dep),
# interleaved so consecutive matmuls rotate PE column groups (e % 4)
REDUCE_ORDER = [10, 11, 0, 1, 12, 13, 2, 3, 14, 15, 4, 5, 6, 7, 8, 9]

_WAITSPLIT_CTR = [0]


def _split_excess_waits(nc, limit=1):
    """walrus (CoreV3) accepts at most `limit` sync-wait commands per
    instruction; Tile can emit more (e.g. the tail drain). Move excess waits
    onto freshly inserted same-engine NoOps, which is semantically identical."""
    n = 0
    for f in nc.m.functions:
        for bb in f.blocks:
            insts = list(bb.instructions)
            out = []
            changed = False
            for inst in insts:
                si = inst.sync_info
                waits = list(si.on_wait) if si else []
                if len(waits) > limit:
                    head, tail = waits[:-limit], waits[-limit:]
                    for i in range(0, len(head), limit):
                        _WAITSPLIT_CTR[0] += 1
                        nop = mybir.InstNoOp(
                            name=f"waitsplit_nop_{_WAITSPLIT_CTR[0]}", ins=[], outs=[]
                        )
                        nop.engine = inst.engine
                        nop.sync_info = mybir.SyncInfo(
                            on_wait=head[i : i + limit], on_update=[]
                        )
                        out.append(nop)
                        n += 1
                    si.on_wait = tail
                    inst.sync_info = si
                    changed = True
                out.append(inst)
            if changed:
                bb.instructions = out
    return n


def _build_nc(S_c):
    nc = bass.Bass()

    tok_pk = nc.dram_tensor("tok_pk", [P, NH * S_c], dt.bfloat16, kind="ExternalInput")
    wt_pk = nc.dram_tensor("wt_pk", [P, NH * NH * P], dt.bfloat16, kind="ExternalInput")
    we_pk = nc.dram_tensor("we_pk", [P, NH * NH * P], dt.bfloat16, kind="ExternalInput")
    ent_pk = nc.dram_tensor("ent_pk", [P, NH * E], dt.bfloat16, kind="ExternalInput")
    sgn_pk = nc.dram_tensor("sgn_pk", [P, NH], dt.float16, kind="ExternalInput")
    bb_pk = nc.dram_tensor("bb_pk", [P, NH], dt.float32, kind="ExternalInput")

    # out[q, g, c]: entity e = 4q+g; c = [cls | p] each S_c wide
    out_t = nc.dram_tensor("out", [NQ, 4, 2 * S_c], dt.float16, kind="ExternalOutput")

    with tile.TileContext(nc) as tc:
        with (
            tc.tile_pool(name="const", bufs=1) as cpool,
            tc.tile_pool(name="wts", bufs=1) as wpool,
            tc.tile_pool(name="usb", bufs=NH) as upool,
            tc.tile_pool(name="acts", bufs=48) as apool,
            tc.tile_pool(name="outs", bufs=1) as opool,
            tc.tile_pool(name="psw", bufs=1, space="PSUM") as psw,
            tc.tile_pool(name="psv", bufs=1, space="PSUM") as psv,
            tc.tile_pool(name="psu", bufs=2, space="PSUM") as psu,
            tc.tile_pool(name="psc", bufs=1, space="PSUM") as psc,
        ):
            # ---- SBUF tiles ----
            t_sgn = cpool.tile([P, NH], dt.float16, tag="sgn")
            t_bb = cpool.tile([P, NH], dt.float32, tag="bb")
            t_dmy = cpool.tile([P, 1], dt.float32, tag="dmy")
            t_wscr = cpool.tile([P, 256], dt.float16, tag="wscr")
            v_sb = cpool.tile([P, NH * E], dt.float32, tag="vsb")
            t_wt = wpool.tile([P, NH * NH * P], dt.bfloat16, tag="wt")
            t_we = wpool.tile([P, NH * NH * P], dt.bfloat16, tag="we")
            t_tok = wpool.tile([P, NH * S_c], dt.bfloat16, tag="tok")
            t_ent = wpool.tile([P, NH * E], dt.bfloat16, tag="ent")
            osb = opool.tile([P, NQ * 2 * S_c], dt.float16, tag="osb")

            # ---- DMA issue. ACT queue: consts + We (j-sliced) ----
            nc.scalar.dma_start(t_sgn[:], sgn_pk[:])
            nc.scalar.dma_start(t_bb[:], bb_pk[:])
            for j in range(3):
                sl = slice(j * NH * P, (j + 1) * NH * P)
                nc.scalar.dma_start(t_we[:, sl], we_pk[:, sl])
            # SP queue: token + Wt (j-sliced)
            nc.sync.dma_start(t_tok[:], tok_pk[:])
            for j in range(NH):
                sl = slice(j * NH * P, (j + 1) * NH * P)
                nc.sync.dma_start(t_wt[:, sl], wt_pk[:, sl])
            # GpSimd queue (SWDGE): scratch init, entities, tail of We
            nc.gpsimd.memset(t_wscr[:], 0.0)
            nc.gpsimd.dma_start(t_ent[:], ent_pk[:])
            for j in range(3, NH):
                sl = slice(j * NH * P, (j + 1) * NH * P)
                nc.gpsimd.dma_start(t_we[:, sl], we_pk[:, sl])

            # dummy sigmoid pulls the ACT table load off the critical path
            nc.scalar.activation(t_dmy[:], t_bb[:, 0:1], AF.Sigmoid)

            # ---- PE warmup (p-state ramp) on a scratch PSUM bank ----
            ps_w = psw.tile([P, 256], dt.float32, tag="wps")
            for w in range(N_WARMUP):
                nc.tensor.matmul(
                    ps_w[0:1, :], t_sgn[:, 0:1], t_wscr[:], start=True, stop=True
                )

            # ---- main pipeline ----
            ps_v = psv.tile([P, NH * E], dt.float32, tag="vps")
            u_sb = [None] * NH
            act_t = [[None] * E for _ in range(NH)]
            ps_c = [
                psc.tile([P, S_c], dt.float32, tag=f"cq{q}", name=f"ps_c{q}")
                for q in range(NQ)
            ]

            def vproj(j):
                # v''T[o in chunk j, e] accumulated over k; bias added on copy
                for k in range(NH):
                    nc.tensor.matmul(
                        ps_v[:, j * E : (j + 1) * E],
                        t_we[:, (j * NH + k) * P : (j * NH + k + 1) * P],
                        t_ent[:, k * E : (k + 1) * E],
                        start=(k == 0),
                        stop=(k == NH - 1),
                    )
                nc.vector.tensor_scalar(
                    v_sb[:, j * E : (j + 1) * E],
                    ps_v[:, j * E : (j + 1) * E],
                    t_bb[:, j : j + 1],
                    None,
                    op0=ALU.add,
                )

            ps_u_t = [None] * NH

            def uproj(j):
                ps_u = psu.tile([P, S_c], dt.float32, tag="ups", name=f"ps_u{j}")
                ps_u_t[j] = ps_u
                for k in range(NH):
                    nc.tensor.matmul(
                        ps_u[:],
                        t_wt[:, (j * NH + k) * P : (j * NH + k + 1) * P],
                        t_tok[:, k * S_c : (k + 1) * S_c],
                        start=(k == 0),
                        stop=(k == NH - 1),
                    )
                u_sb[j] = upool.tile([P, S_c], dt.float16, tag="u", name=f"u_sb{j}")
                nc.scalar.copy(u_sb[j][:], ps_u[:])

            def acts(j):
                # ScalarE entities first (read PSUM directly; no u-copy dep)
                for e in range(N_DVE, E):
                    a = apool.tile([P, S_c], dt.float16, tag="act", name=f"act_{j}_{e}")
                    act_t[j][e] = a
                    bias = v_sb[:, j * E + e : j * E + e + 1]
                    nc.scalar.activation(a[:], ps_u_t[j][:], AF.Relu, bias=bias)
                for e in range(N_DVE):
                    a = apool.tile([P, S_c], dt.float16, tag="act", name=f"act_{j}_{e}")
                    act_t[j][e] = a
                    bias = v_sb[:, j * E + e : j * E + e + 1]
                    nc.vector.tensor_scalar(
                        a[:], u_sb[j][:], bias, 0.0, op0=ALU.add, op1=ALU.max
                    )

            def reduce(j):
                for e in REDUCE_ORDER:
                    q, g = e // 4, e % 4
                    nc.tensor.matmul(
                        ps_c[q][32 * g : 32 * g + 1, :],
                        t_sgn[:, j : j + 1],
                        act_t[j][e][:],
                        start=(j == 0),
                        stop=(j == NH - 1),
                        tile_position=(0, 32 * g),
                    )

            # PE program order: warmup, then per-j v/u interleaved with
            # reduce lagging one chunk.
            vproj(0)
            uproj(0)
            acts(0)
            vproj(1)
            uproj(1)
            acts(1)
            reduce(0)
            for j in range(2, NH):
                vproj(j)
                uproj(j)
                acts(j)
                reduce(j - 1)
            reduce(NH - 1)

            # ---- tail: cls copy (DVE) + sigmoid (ACT) per quad, one out DMA
            for q in range(NQ):
                nc.vector.tensor_scalar(
                    osb[:, q * 2 * S_c : q * 2 * S_c + S_c],
                    ps_c[q][:],
                    0.0,
                    None,
                    op0=ALU.add,
                )
                nc.scalar.activation(
                    osb[:, q * 2 * S_c + S_c : (q + 1) * 2 * S_c],
                    ps_c[q][:],
                    AF.Sigmoid,
                )
            try:
                src = osb[0 : P : 32, :].rearrange("p (q c) -> p q c", q=NQ)
                dst = out_t[:].rearrange("q g c -> g q c")
                nc.sync.dma_start(dst, src)
            except Exception:
                for q in range(NQ):
                    nc.sync.dma_start(
                        out_t[q, :, :],
                        osb[0 : P : 32, q * 2 * S_c : (q + 1) * 2 * S_c],
                    )

    _split_excess_waits(nc, limit=1)
    return nc


_NC_CACHE = {}


def _get_nc(S_c):
    if S_c not in _NC_CACHE:
        _NC_CACHE[S_c] = _build_nc(S_c)
    return _NC_CACHE[S_c]


def _pack_pmajor(mat, ncols):
    """[H, ncols] -> [P, NH*ncols] partition-major: out[p, k*ncols+c] =
    mat[k*P+p, c]."""
    return np.ascontiguousarray(
        mat.reshape(NH, P, ncols).transpose(1, 0, 2).reshape(P, NH * ncols)
    )


def kernel(token_embedding, entity_embedding, token_mask, Wt, bt, We, be, wb, **kw):
    token_embedding = np.asarray(token_embedding, dtype=np.float32)
    entity_embedding = np.asarray(entity_embedding, dtype=np.float32)
    token_mask = np.asarray(token_mask).astype(bool)
    Wt = np.asarray(Wt, dtype=np.float32)
    bt = np.asarray(bt, dtype=np.float32)
    We = np.asarray(We, dtype=np.float32)
    be = np.asarray(be, dtype=np.float32)
    wb = np.asarray(wb, dtype=np.float32)

    bf16 = ml_dtypes.bfloat16

    a = np.abs(wb)
    sgn = np.where(wb >= 0, np.float32(1.0), np.float32(-1.0))

    # fold |wb| into the weights; transpose to [h, o]
    W2t = (Wt * a[:, None]).T.astype(np.float32)  # [h, o]
    W2e = (We * a[:, None]).T.astype(np.float32)
    bb = ((bt + be) * a).astype(np.float32)

    # wt_pk[p, (j*NH+k)*P + c] = W2[k*P+p, j*P+c]  (j-major blocks)
    def pack_w(W2):
        arr = W2.reshape(NH, P, NH, P).transpose(1, 2, 0, 3)  # [p, j, k, c]
        return np.ascontiguousarray(arr.reshape(P, NH * NH * P)).astype(bf16)

    wt_pk = pack_w(W2t)
    we_pk = pack_w(W2e)
    sgn_pk = np.ascontiguousarray(sgn.reshape(NH, P).T).astype(np.float16)
    bb_pk = np.ascontiguousarray(bb.reshape(NH, P).T).astype(np.float32)

    idxs = [np.nonzero(token_mask[b])[0] for b in range(B)]
    nmax = max((len(ix) for ix in idxs), default=1)
    S_c = max(64, -(-nmax // 64) * 64)

    nc = _get_nc(S_c)
    in_maps = []
    for b in range(B):
        ix = idxs[b]
        tokc = np.zeros((S_c, H), dtype=np.float32)
        tokc[: len(ix)] = token_embedding[b][ix]
        tok_pk = _pack_pmajor(tokc.T, S_c).astype(bf16)  # [P, NH*S_c]
        ent_pk = _pack_pmajor(entity_embedding[b].T, E).astype(bf16)
        in_maps.append(
            {
                "tok_pk": tok_pk,
                "wt_pk": wt_pk,
                "we_pk": we_pk,
                "ent_pk": ent_pk,
                "sgn_pk": sgn_pk,
                "bb_pk": bb_pk,
            }
        )

    res = run_bass_kernel_spmd(nc, in_maps, core_ids=list(range(B)))

    cls = np.full((B, E, S), -10000.0, dtype=np.float32)
    p = np.zeros((B, E, S), dtype=np.float32)
    for b in range(B):
        o = np.asarray(res.results[b]["out"], dtype=np.float32).reshape(E, 2 * S_c)
        ix = idxs[b]
        cls[b][:, ix] = o[:, : len(ix)]
        p[b][:, ix] = o[:, S_c : S_c + len(ix)]
    return cls, p


# revision 10
# speedup vs baseline: 1.5279x; 1.0177x over previous
"""Trainium2 Bass kernel for HeadTailBoundaryPredictor.

Reference computation (B=8, S=512, E=16, H=768):
    t   = token @ Wt.T + bt                    [B,S,H]
    e2  = ent @ We.T + be                      [B,E,H]
    cls = einsum('besh,h->bes', relu(t[:,None]+e2[:,:,None]), wb)
    cls = where(mask, cls, -1e4); p = sigmoid(cls)

Math restructure: fold wb into the projections. With a = |wb|, s = sign(wb):
    cls[e,s] = sum_o s[o] * relu( a[o]*t[s,o] + a[o]*e2[e,o] )
since a[o]*relu(x) = relu(a[o]*x) for a >= 0.

Device plan (per core = one batch, data-parallel over B):
  - Host compacts the sequence dim: only token positions with mask=1 are
    shipped/computed (S_c = roundup(max_count, 64)); masked outputs are the
    constants -1e4 / sigmoid(-1e4)=0, filled host-side.
  - token/Wt/We/ent are bf16 (halves DMA); u/acts are f16 so the DVE runs
    tensor_scalar in 4x mode; reduce matmuls are f16 (full PE rate).
  - u'T[o,s]  = (diag(a) Wt tokenT)   per o-chunk j, bf16 matmuls (TensorE)
  - v''T[o,e] = (diag(a) We entT) + a*(bt+be)   (TensorE + bias add)
  - act[o,s]  = relu(u' + v''[:,e]) f16, per entity: 13 on VectorE (4x mode),
    2 on ScalarE, 1 on GpSimd
  - cls[e,s]  = sgnT @ act   via 1-column f16 matmuls rotated over the 4
    PE column groups (tile_position) so up to 4 run concurrently
  - p = sigmoid(cls) (ScalarE); outputs f16, host casts/scatters.
  - All DRAM inputs are host-packed partition-major so each is a single
    contiguous-per-partition DMA; descriptor gen is split across the
    SP and ACT hardware DGE queues (weights j-sliced to unblock compute).
"""

import sys

for _p in ("/opt/trn_rl_repo", "/root/.axon_site/_ro/trn_rl_repo"):
    if _p not in sys.path:
        sys.path.append(_p)

import numpy as np
import ml_dtypes

import concourse.bass as bass
import concourse.mybir as mybir
import concourse.tile as tile
from concourse.bass_utils import run_bass_kernel_spmd

dt = mybir.dt
AF = mybir.ActivationFunctionType
ALU = mybir.AluOpType

B, S, E, H = 8, 512, 16, 768
P = 128
NH = H // P  # 6 chunks of the hidden/output dims
NQ = E // 4  # 4 entity quads (one PSUM bank each)

N_WARMUP = 4
N_DVE = 11  # entities 0..10 on VectorE (from u_sb f16); 11..15 on ScalarE (from PSUM)
# reduce consumption order: ScalarE entities are ready first (no u-copy dep),
# interleaved so consecutive matmuls rotate PE column groups (e % 4)
REDUCE_ORDER = [11, 12, 13, 14, 15, 0, 1, 2, 3, 4, 5, 6, 7, 8, 9, 10]

_WAITSPLIT_CTR = [0]


def _split_excess_waits(nc, limit=1):
    """walrus (CoreV3) accepts at most `limit` sync-wait commands per
    instruction; Tile can emit more (e.g. the tail drain). Move excess waits
    onto freshly inserted same-engine NoOps, which is semantically identical."""
    n = 0
    for f in nc.m.functions:
        for bb in f.blocks:
            insts = list(bb.instructions)
            out = []
            changed = False
            for inst in insts:
                si = inst.sync_info
                waits = list(si.on_wait) if si else []
                if len(waits) > limit:
                    head, tail = waits[:-limit], waits[-limit:]
                    for i in range(0, len(head), limit):
                        _WAITSPLIT_CTR[0] += 1
                        nop = mybir.InstNoOp(
                            name=f"waitsplit_nop_{_WAITSPLIT_CTR[0]}", ins=[], outs=[]
                        )
                        nop.engine = inst.engine
                        nop.sync_info = mybir.SyncInfo(
                            on_wait=head[i : i + limit], on_update=[]
                        )
                        out.append(nop)
                        n += 1
                    si.on_wait = tail
                    inst.sync_info = si
                    changed = True
                out.append(inst)
            if changed:
                bb.instructions = out
    return n


def _build_nc(S_c):
    nc = bass.Bass()

    tok_pk = nc.dram_tensor("tok_pk", [P, NH * S_c], dt.bfloat16, kind="ExternalInput")
    wt_pk = nc.dram_tensor("wt_pk", [P, NH * NH * P], dt.bfloat16, kind="ExternalInput")
    we_pk = nc.dram_tensor("we_pk", [P, NH * NH * P], dt.bfloat16, kind="ExternalInput")
    ent_pk = nc.dram_tensor("ent_pk", [P, NH * E], dt.bfloat16, kind="ExternalInput")
    sgn_pk = nc.dram_tensor("sgn_pk", [P, NH], dt.float16, kind="ExternalInput")
    bb_pk = nc.dram_tensor("bb_pk", [P, NH], dt.float32, kind="ExternalInput")

    # out[q, g, c]: entity e = 4q+g; c = [cls | p] each S_c wide
    out_t = nc.dram_tensor("out", [NQ, 4, 2 * S_c], dt.float16, kind="ExternalOutput")

    with tile.TileContext(nc) as tc:
        with (
            tc.tile_pool(name="const", bufs=1) as cpool,
            tc.tile_pool(name="wts", bufs=1) as wpool,
            tc.tile_pool(name="usb", bufs=NH) as upool,
            tc.tile_pool(name="acts", bufs=48) as apool,
            tc.tile_pool(name="outs", bufs=1) as opool,
            tc.tile_pool(name="psw", bufs=1, space="PSUM") as psw,
            tc.tile_pool(name="psv", bufs=1, space="PSUM") as psv,
            tc.tile_pool(name="psu", bufs=2, space="PSUM") as psu,
            tc.tile_pool(name="psc", bufs=1, space="PSUM") as psc,
        ):
            # ---- SBUF tiles ----
            t_sgn = cpool.tile([P, NH], dt.float16, tag="sgn")
            t_bb = cpool.tile([P, NH], dt.float32, tag="bb")
            t_dmy = cpool.tile([P, 1], dt.float32, tag="dmy")
            t_wscr = cpool.tile([P, 256], dt.float16, tag="wscr")
            v_sb = cpool.tile([P, NH * E], dt.float32, tag="vsb")
            t_wt = wpool.tile([P, NH * NH * P], dt.bfloat16, tag="wt")
            t_we = wpool.tile([P, NH * NH * P], dt.bfloat16, tag="we")
            t_tok = wpool.tile([P, NH * S_c], dt.bfloat16, tag="tok")
            t_ent = wpool.tile([P, NH * E], dt.bfloat16, tag="ent")
            osb = opool.tile([P, NQ * 2 * S_c], dt.float16, tag="osb")

            # ---- DMA issue. ACT queue: consts + We (j-sliced) ----
            nc.scalar.dma_start(t_sgn[:], sgn_pk[:])
            nc.scalar.dma_start(t_bb[:], bb_pk[:])
            for j in range(3):
                sl = slice(j * NH * P, (j + 1) * NH * P)
                nc.scalar.dma_start(t_we[:, sl], we_pk[:, sl])
            # SP queue: token + Wt (j-sliced)
            nc.sync.dma_start(t_tok[:], tok_pk[:])
            for j in range(NH):
                sl = slice(j * NH * P, (j + 1) * NH * P)
                nc.sync.dma_start(t_wt[:, sl], wt_pk[:, sl])
            # GpSimd queue (SWDGE): scratch init, entities, tail of We
            nc.gpsimd.memset(t_wscr[:], 0.0)
            nc.gpsimd.dma_start(t_ent[:], ent_pk[:])
            for j in range(3, NH):
                sl = slice(j * NH * P, (j + 1) * NH * P)
                nc.gpsimd.dma_start(t_we[:, sl], we_pk[:, sl])

            # dummy sigmoid pulls the ACT table load off the critical path
            nc.scalar.activation(t_dmy[:], t_bb[:, 0:1], AF.Sigmoid)

            # ---- PE warmup (p-state ramp) on a scratch PSUM bank ----
            ps_w = psw.tile([P, 256], dt.float32, tag="wps")
            for w in range(N_WARMUP):
                nc.tensor.matmul(
                    ps_w[0:1, :], t_sgn[:, 0:1], t_wscr[:], start=True, stop=True
                )

            # ---- main pipeline ----
            ps_v = psv.tile([P, NH * E], dt.float32, tag="vps")
            u_sb = [None] * NH
            act_t = [[None] * E for _ in range(NH)]
            ps_c = [
                psc.tile([P, S_c], dt.float32, tag=f"cq{q}", name=f"ps_c{q}")
                for q in range(NQ)
            ]

            def vproj(j):
                # v''T[o in chunk j, e] accumulated over k; bias added on copy
                for k in range(NH):
                    nc.tensor.matmul(
                        ps_v[:, j * E : (j + 1) * E],
                        t_we[:, (j * NH + k) * P : (j * NH + k + 1) * P],
                        t_ent[:, k * E : (k + 1) * E],
                        start=(k == 0),
                        stop=(k == NH - 1),
                    )
                nc.vector.tensor_scalar(
                    v_sb[:, j * E : (j + 1) * E],
                    ps_v[:, j * E : (j + 1) * E],
                    t_bb[:, j : j + 1],
                    None,
                    op0=ALU.add,
                )

            ps_u_t = [None] * NH

            def uproj(j):
                ps_u = psu.tile([P, S_c], dt.float32, tag="ups", name=f"ps_u{j}")
                ps_u_t[j] = ps_u
                for k in range(NH):
                    nc.tensor.matmul(
                        ps_u[:],
                        t_wt[:, (j * NH + k) * P : (j * NH + k + 1) * P],
                        t_tok[:, k * S_c : (k + 1) * S_c],
                        start=(k == 0),
                        stop=(k == NH - 1),
                    )
                u_sb[j] = upool.tile([P, S_c], dt.float16, tag="u", name=f"u_sb{j}")
                nc.scalar.copy(u_sb[j][:], ps_u[:])

            def acts(j):
                # ScalarE entities first (read PSUM directly; no u-copy dep)
                for e in range(N_DVE, E):
                    a = apool.tile([P, S_c], dt.float16, tag="act", name=f"act_{j}_{e}")
                    act_t[j][e] = a
                    bias = v_sb[:, j * E + e : j * E + e + 1]
                    nc.scalar.activation(a[:], ps_u_t[j][:], AF.Relu, bias=bias)
                for e in range(N_DVE):
                    a = apool.tile([P, S_c], dt.float16, tag="act", name=f"act_{j}_{e}")
                    act_t[j][e] = a
                    bias = v_sb[:, j * E + e : j * E + e + 1]
                    nc.vector.tensor_scalar(
                        a[:], u_sb[j][:], bias, 0.0, op0=ALU.add, op1=ALU.max
                    )

            def reduce(j):
                for e in REDUCE_ORDER:
                    q, g = e // 4, e % 4
                    nc.tensor.matmul(
                        ps_c[q][32 * g : 32 * g + 1, :],
                        t_sgn[:, j : j + 1],
                        act_t[j][e][:],
                        start=(j == 0),
                        stop=(j == NH - 1),
                        tile_position=(0, 32 * g),
                    )

            # PE program order: warmup, then per-j u/v interleaved with
            # reduce lagging one chunk. u-proj first: its DMA deps (tok+wt)
            # land before v-proj's (we+ent), so the PE starts sooner.
            uproj(0)
            vproj(0)
            acts(0)
            uproj(1)
            vproj(1)
            acts(1)
            reduce(0)
            for j in range(2, NH):
                uproj(j)
                vproj(j)
                acts(j)
                reduce(j - 1)

            # last chunk: finish one quad at a time and start its tail
            # (cls copy on DVE, sigmoid on ACT) while later quads reduce.
            j = NH - 1
            for q in range(NQ):
                for g in range(4):
                    e = 4 * q + g
                    nc.tensor.matmul(
                        ps_c[q][32 * g : 32 * g + 1, :],
                        t_sgn[:, j : j + 1],
                        act_t[j][e][:],
                        start=False,
                        stop=True,
                        tile_position=(0, 32 * g),
                    )
                nc.vector.tensor_scalar(
                    osb[:, q * 2 * S_c : q * 2 * S_c + S_c],
                    ps_c[q][:],
                    0.0,
                    None,
                    op0=ALU.add,
                )
                nc.scalar.activation(
                    osb[:, q * 2 * S_c + S_c : (q + 1) * 2 * S_c],
                    ps_c[q][:],
                    AF.Sigmoid,
                )
            try:
                src = osb[0 : P : 32, :].rearrange("p (q c) -> p q c", q=NQ)
                dst = out_t[:].rearrange("q g c -> g q c")
                nc.sync.dma_start(dst, src)
            except Exception:
                for q in range(NQ):
                    nc.sync.dma_start(
                        out_t[q, :, :],
                        osb[0 : P : 32, q * 2 * S_c : (q + 1) * 2 * S_c],
                    )

    _split_excess_waits(nc, limit=1)
    return nc


_NC_CACHE = {}


def _get_nc(S_c):
    if S_c not in _NC_CACHE:
        _NC_CACHE[S_c] = _build_nc(S_c)
    return _NC_CACHE[S_c]


def _pack_pmajor(mat, ncols):
    """[H, ncols] -> [P, NH*ncols] partition-major: out[p, k*ncols+c] =
    mat[k*P+p, c]."""
    return np.ascontiguousarray(
        mat.reshape(NH, P, ncols).transpose(1, 0, 2).reshape(P, NH * ncols)
    )


def kernel(token_embedding, entity_embedding, token_mask, Wt, bt, We, be, wb, **kw):
    token_embedding = np.asarray(token_embedding, dtype=np.float32)
    entity_embedding = np.asarray(entity_embedding, dtype=np.float32)
    token_mask = np.asarray(token_mask).astype(bool)
    Wt = np.asarray(Wt, dtype=np.float32)
    bt = np.asarray(bt, dtype=np.float32)
    We = np.asarray(We, dtype=np.float32)
    be = np.asarray(be, dtype=np.float32)
    wb = np.asarray(wb, dtype=np.float32)

    bf16 = ml_dtypes.bfloat16

    a = np.abs(wb)
    sgn = np.where(wb >= 0, np.float32(1.0), np.float32(-1.0))

    # fold |wb| into the weights; transpose to [h, o]
    W2t = (Wt * a[:, None]).T.astype(np.float32)  # [h, o]
    W2e = (We * a[:, None]).T.astype(np.float32)
    bb = ((bt + be) * a).astype(np.float32)

    # wt_pk[p, (j*NH+k)*P + c] = W2[k*P+p, j*P+c]  (j-major blocks)
    def pack_w(W2):
        arr = W2.reshape(NH, P, NH, P).transpose(1, 2, 0, 3)  # [p, j, k, c]
        return np.ascontiguousarray(arr.reshape(P, NH * NH * P)).astype(bf16)

    wt_pk = pack_w(W2t)
    we_pk = pack_w(W2e)
    sgn_pk = np.ascontiguousarray(sgn.reshape(NH, P).T).astype(np.float16)
    bb_pk = np.ascontiguousarray(bb.reshape(NH, P).T).astype(np.float32)

    idxs = [np.nonzero(token_mask[b])[0] for b in range(B)]
    nmax = max((len(ix) for ix in idxs), default=1)
    S_c = max(64, -(-nmax // 32) * 32)

    nc = _get_nc(S_c)
    in_maps = []
    for b in range(B):
        ix = idxs[b]
        tokc = np.zeros((S_c, H), dtype=np.float32)
        tokc[: len(ix)] = token_embedding[b][ix]
        tok_pk = _pack_pmajor(tokc.T, S_c).astype(bf16)  # [P, NH*S_c]
        ent_pk = _pack_pmajor(entity_embedding[b].T, E).astype(bf16)
        in_maps.append(
            {
                "tok_pk": tok_pk,
                "wt_pk": wt_pk,
                "we_pk": we_pk,
                "ent_pk": ent_pk,
                "sgn_pk": sgn_pk,
                "bb_pk": bb_pk,
            }
        )

    res = run_bass_kernel_spmd(nc, in_maps, core_ids=list(range(B)))

    cls = np.full((B, E, S), -10000.0, dtype=np.float32)
    p = np.zeros((B, E, S), dtype=np.float32)
    for b in range(B):
        o = np.asarray(res.results[b]["out"], dtype=np.float32).reshape(E, 2 * S_c)
        ix = idxs[b]
        cls[b][:, ix] = o[:, : len(ix)]
        p[b][:, ix] = o[:, S_c : S_c + len(ix)]
    return cls, p
